# revision 62
# baseline (speedup 1.0000x reference)
"""CLUB mutual-information upper bound (loss_fn) on 8 Trainium2 NeuronCores.

Math: the reference computes
    h  = relu(x1 @ W1 + b1); h = relu(h @ W2 + b2); g = tanh(h @ W3 + b3)
    mu, logvar = split(g); iv = exp(-logvar)
    pos = -0.5 (mu - x2)^2 iv
    neg = -0.5 mean_j[(mu_i - x2_j)^2] iv     (the [N,N,D] pairwise term)
    mi  = mean_i sum_d (pos - neg)
With m1 = mean_j x2, m2 = mean_j x2^2 (host-computed, global over all N),
the pairwise term collapses:  pos - neg = iv (mu*A - B) where A = x2 - m1,
B = 0.5 (x2^2 - m2). Each of the 8 cores runs its 128-row shard through the
MLP and accumulates c1_d = sum_i iv*mu*A, c2_d = sum_i iv*B per feature
partition; the host finishes mi = sum_cores sum_d (c1 - c2) / N.

Measured-window model (from neuron-profile traces): exec_time =
(body barrier end - first framework MEMSET) + ~7.39us fixed tail (the
postamble resets ~250 hw semaphores serially before the completion notify).
Only the body is optimizable; this version's body is ~8.5-8.8us.

Structure (each element validated against trace anchors):
  - One fp16 HWDGE stream on the SP ring: [bias|x1T|W1k0] (dw1a, gates the
    first matmuls) -> [W1k1] (dw1b) -> [W2k0] -> [W3k0]. The ring streams
    only ~140-165GB/s, so the k1 halves of W2/W3 ride the parallel SWDGE
    path (w2k1 -> brow -> w3k1 -> A|B). DMA completion = data end + ~900ns
    sem-prop, plus a ~45ns/inc 16-increment train serialized per queue.
  - Biases: b1/b2 as f32 bit-packed fp16 column pairs applied via the ACT
    relu bias operand (m1 halves) and DVE tensor_scalar scalar-AP add (m0
    halves); b3 via outer-product matmuls (1-partition rows x ones) that
    open the two L3 psum banks during the relu2 idle window.
  - Layer matmuls split k(contraction)/m(feature-half); relu m0 on DVE and
    m1 on ACT run in parallel; per-layer rhs comes from the previous
    layer's two relu outputs, so k0 matmuls start as soon as the DVE half
    lands.
  - L3: logvar accumulates in ps3m0 and closes one matmul earlier than mu
    (k-matmul order k0lv, k1lv(close), k0mu, k1mu), so the serial scalar
    chain tanh(lv) -> exp starts ~200ns sooner; tanh(mu) then reads ps3m1
    without a semaphore wait (it retires <=220ns after ps3m0 closes while
    tanh+exp occupy scalar >=650ns).
  - Tail on DVE: c2 = iv*B (at exp), tmp = mu*A (at mu-tanh), c1 = iv*tmp,
    with per-partition accumulators into out[128,2].
  - The out DMA is released on the exp semaphore: its ~640ns issue plus the
    idle-queue first-read latency covers the trailing writes, and the fixed
    postamble covers the flight. Warmup dummy matmuls keep the PE busy
    until dw1a lands.

Known-negative variants (all measured): x1 or W-halves first on SWDGE (the
Pool engine issues ~650ns late), dual-ring HWDGE streaming (rings share
bandwidth), merged single-bank L3 psum (group serialization), fp16 A/B
(costs 7e-3 accuracy), earlier out-DMA release (races the accum writes).
"""
import sys
from contextlib import ExitStack

import numpy as np

sys.path.insert(0, "/opt/trn_rl_repo")

import concourse.bass as bass
from concourse import mybir
from concourse.bass_utils import run_bass_kernel_spmd

F32 = mybir.dt.float32
F16 = mybir.dt.float16
NCORES = 8
N = 1024
X1D = 256
X2D = 128
HID = 256
ROWS = N // NCORES  # 128
P = 128

# blobW (fp16) [128, 1808]:
#   [0:16)       bias cols: f32 values bit-packed as fp16 pairs;
#                pair (2l+m)*2 = b_{l+1}[m*128:(m+1)*128]; pair 4 = zeros
#                (AP biases everywhere keep the framework from emitting
#                const-AP memsets in the preamble)
#   [16:272)     x1T   col 16 + k*128 + j = x1s[j, k*128+p]
#   [272:784)    W1    col 272 + k*256 + m*128 + j = W1[k*128+p, m*128+j]
#                (k-major so [bias|x1|W1k0] is one contiguous DMA -> dw1a,
#                 W1k1 a second -> dw1b: k0 matmuls start one DMA earlier)
#   [784:1040)   W2 k0 only
#   [1040:1296)  W3 k0 only — the k1 halves of W2 and W3 ride the SWDGE
#                path: the HWDGE ring streams only ~140-165GB/s, so
#                shedding 132KB pulls dw2 in before relu1 finishes
# blobW2K1/blobW3K1 (fp16) [128, 256]: k1 halves, col m*128+j = W[128+p, m*128+j]
# brow (fp16) [1, 384]: [0:128) b3m0, [128:256) b3m1, [256:384) ones
# blobAB (fp32) [128, 256]: [0:128) A = (x2s - m1).T ; [128:256) B = 0.5(x2s^2 - m2).T
#   (f32: the final value is a small difference of large sums, fp16 A/B cost
#    ~7e-3 of relative error on the result)
BIAS_OFF = 0
X1_OFF = 16
W1_OFF = 272
W1K1_OFF = 528
W2_OFF = 784
W3_OFF = 1040
BW_W = 1296

N_DUMMIES = 18
N_NARROW = 4

_module_cache = None


def _build_module(n_dummies=N_DUMMIES, n_narrow=N_NARROW):
    nc = bass.Bass()
    blobW = nc.declare_dram_parameter("blobW", [P, BW_W], F16, isOutput=False)
    blobW2K1 = nc.declare_dram_parameter("blobW2K1", [P, 256], F16, isOutput=False)
    blobW3K1 = nc.declare_dram_parameter("blobW3K1", [P, 256], F16, isOutput=False)
    brow = nc.declare_dram_parameter("brow", [1, 384], F16, isOutput=False)
    blobAB = nc.declare_dram_parameter("blobAB", [P, 256], F32, isOutput=False)
    out = nc.declare_dram_parameter("out", [P, 2], F32, isOutput=True)

    AF = mybir.ActivationFunctionType
    ALU = mybir.AluOpType

    with ExitStack() as ctx:
        ec = ctx.enter_context
        bw = ec(nc.sbuf_tensor("bw", [P, BW_W], F16))
        w2k1 = ec(nc.sbuf_tensor("w2k1", [P, 256], F16))
        w3k1 = ec(nc.sbuf_tensor("w3k1", [P, 256], F16))
        brow_sb = ec(nc.sbuf_tensor("brow_sb", [1, 384], F16))
        ab = ec(nc.sbuf_tensor("ab", [P, 256], F32))
        h1m0 = ec(nc.sbuf_tensor("h1m0", [P, ROWS], F16))
        h1m1 = ec(nc.sbuf_tensor("h1m1", [P, ROWS], F16))
        h2m0 = ec(nc.sbuf_tensor("h2m0", [P, ROWS], F16))
        h2m1 = ec(nc.sbuf_tensor("h2m1", [P, ROWS], F16))
        g = ec(nc.sbuf_tensor("g", [P, 2 * ROWS], F32))  # mu | logvar->tanh
        iv = ec(nc.sbuf_tensor("iv", [P, ROWS], F32))
        tmp = ec(nc.sbuf_tensor("tmp", [P, ROWS], F32))
        scr = ec(nc.sbuf_tensor("scr", [P, ROWS], F32))
        out_sb = ec(nc.sbuf_tensor("out_sb", [P, 2], F32))
        ps1m0 = ec(nc.psum_tensor("ps1m0", [P, ROWS], F32))
        ps1m1 = ec(nc.psum_tensor("ps1m1", [P, ROWS], F32))
        ps2m0 = ec(nc.psum_tensor("ps2m0", [P, ROWS], F32))
        ps2m1 = ec(nc.psum_tensor("ps2m1", [P, ROWS], F32))
        ps3m0 = ec(nc.psum_tensor("ps3m0", [P, ROWS], F32))
        ps3m1 = ec(nc.psum_tensor("ps3m1", [P, ROWS], F32))
        psw = ec(nc.psum_tensor("psw", [P, ROWS], F32))
        dbrow = ec(nc.semaphore("dbrow"))
        dw1a = ec(nc.semaphore("dw1a"))
        dw1b = ec(nc.semaphore("dw1b"))
        dw2 = ec(nc.semaphore("dw2"))
        dw3 = ec(nc.semaphore("dw3"))
        dw2k1 = ec(nc.semaphore("dw2k1"))
        dw3k1 = ec(nc.semaphore("dw3k1"))
        dab = ec(nc.semaphore("dab"))
        s_pe = ec(nc.semaphore("s_pe"))
        s_act = ec(nc.semaphore("s_act"))
        s_dve = ec(nc.semaphore("s_dve"))
        dout = ec(nc.semaphore("dout"))
        block = ec(nc.Block())

        x1T = [bw[:, X1_OFF : X1_OFF + 128], bw[:, X1_OFF + 128 : X1_OFF + 256]]
        zcol = bw[:, 8:10].bitcast(F32)  # zero bias pair
        A_ap = ab[:, 0:ROWS]
        B_ap = ab[:, ROWS : 2 * ROWS]

        def w_ap(woff, k, m):
            c = woff + k * 256 + m * 128
            return bw[:, c : c + 128]

        def b_col(l, m):
            c = BIAS_OFF + (2 * l + m) * 2
            return bw[:, c : c + 2].bitcast(F32)

        b3m0_row = brow_sb[0:1, 0:128]
        b3m1_row = brow_sb[0:1, 128:256]
        ones_row = brow_sb[0:1, 256:384]

        mu_ap = g[:, 0:ROWS]
        lv_ap = g[:, ROWS : 2 * ROWS]

        @block.sync
        def _(sync):
            # HWDGE stream: the L1-critical prefix first ([bias|x1|W1k0] then
            # [W1k1] so k0 matmuls start one DMA-chunk earlier), then W2, W3
            # pipelining behind on the same ring.
            sync.dma_start(
                out=bw[:, 0:W1K1_OFF], in_=blobW[:, 0:W1K1_OFF]
            ).then_inc(dw1a, 16)
            sync.dma_start(
                out=bw[:, W1K1_OFF:W2_OFF], in_=blobW[:, W1K1_OFF:W2_OFF]
            ).then_inc(dw1b, 16)
            sync.dma_start(
                out=bw[:, W2_OFF:W3_OFF], in_=blobW[:, W2_OFF:W3_OFF]
            ).then_inc(dw2, 16)
            sync.dma_start(
                out=bw[:, W3_OFF:BW_W], in_=blobW[:, W3_OFF:BW_W]
            ).then_inc(dw3, 16)  # W3 k0 only
            # out DMA released when exp is done (s_act>=4): the ~640ns
            # issue + the idle-queue first-read latency (~770ns) covers the
            # trailing mu-tanh + DVE accum writes with ~600ns margin, and
            # the fixed end-of-kernel semaphore sweep covers the flight.
            sync.wait_ge(s_act, 4)
            sync.dma_start(out=out[:], in_=out_sb[:]).then_inc(dout, 16)

        @block.gpsimd
        def _(gpsimd):
            # SWDGE has its own descriptor generator and runs in parallel
            # with the HWDGE ring: W3's k1 half first (sheds 66KB off the
            # HWDGE stream), then the tiny b3/ones row (needed by the L3
            # bias matmuls mid-chain), then the f32 A|B (tail only).
            # (x1 via SWDGE was tried and lost ~700ns: the Pool engine
            # issues its first DMA ~650ns later than Sync does.)
            gpsimd.dma_start(out=w2k1[:], in_=blobW2K1[:]).then_inc(dw2k1, 16)
            gpsimd.dma_start(out=brow_sb[0:1, :], in_=brow[0:1, :]).then_inc(dbrow, 16)
            gpsimd.dma_start(out=w3k1[:], in_=blobW3K1[:]).then_inc(dw3k1, 16)
            gpsimd.dma_start(out=ab[:], in_=blobAB[:]).then_inc(dab, 16)

        @block.scalar
        def _(scalar):
            # dummy activations: ACT table load (relu/tanh/exp) starts early.
            # All biases are APs (zcol) so the framework emits no const-AP
            # memsets in the preamble (zcol holds garbage this early; the
            # dummy outputs are dead stores into scr).
            scalar.activation(
                out=scr[0:1, 0:1], in_=scr[0:1, 0:1], func=AF.Relu,
                bias=zcol[0:1, 0:1], scale=1.0
            )
            scalar.activation(
                out=scr[0:1, 0:1], in_=scr[0:1, 0:1], func=AF.Tanh,
                bias=zcol[0:1, 0:1], scale=1.0
            )
            scalar.activation(
                out=scr[0:1, 0:1], in_=scr[0:1, 0:1], func=AF.Exp,
                bias=zcol[0:1, 0:1], scale=0.0
            )
            # relu m1 halves (bias via ACT bias operand, fp16 [128,1] AP)
            scalar.wait_ge(s_pe, 2)
            scalar.activation(
                out=h1m1[:], in_=ps1m1[:], func=AF.Relu, bias=b_col(0, 1), scale=1.0
            ).then_inc(s_act)
            scalar.wait_ge(s_pe, 4)
            scalar.activation(
                out=h2m1[:], in_=ps2m1[:], func=AF.Relu, bias=b_col(1, 1), scale=1.0
            ).then_inc(s_act)
            # logvar lives in ps3m0, which closes one matmul earlier than
            # ps3m1 -> the tanh->exp chain starts ~107ns sooner. The mu tanh
            # reads ps3m1 without a wait: k1m1 retires <=110ns after k1m0
            # while tanh_lv+exp occupy scalar >=650ns.
            scalar.wait_ge(s_pe, 5)
            scalar.activation(
                out=lv_ap, in_=ps3m0[:], func=AF.Tanh, bias=zcol, scale=1.0
            ).then_inc(s_act)
            scalar.activation(
                out=iv[:], in_=lv_ap, func=AF.Exp, bias=zcol, scale=-1.0
            ).then_inc(s_act)
            scalar.activation(
                out=mu_ap, in_=ps3m1[:], func=AF.Tanh, bias=zcol, scale=1.0
            ).then_inc(s_act)

        @block.tensor
        def _(tensor):
            # warmup: keep the PE clock ramped while the input DMA flies.
            for _i in range(n_dummies):
                tensor.matmul(psw[:], lhsT=bw[:, 0:128], rhs=bw[:, 0:128],
                              start=True, stop=True)
            for _i in range(n_narrow):
                tensor.matmul(psw[:, 0:32], lhsT=bw[:, 0:128],
                              rhs=bw[:, 0:32], start=True, stop=True)
            # L1: bias comes via the relu, so psum groups start at k0.
            # [bias|x1|W1k0] arrive one HWDGE chunk before W1k1.
            tensor.wait_ge(dw1a, 16)
            tensor.matmul(ps1m0[:], lhsT=w_ap(W1_OFF, 0, 0), rhs=x1T[0], start=True, stop=False)
            tensor.matmul(ps1m1[:], lhsT=w_ap(W1_OFF, 0, 1), rhs=x1T[0], start=True, stop=False)
            tensor.wait_ge(dw1b, 16)
            tensor.matmul(ps1m0[:], lhsT=w_ap(W1_OFF, 1, 0), rhs=x1T[1], start=False, stop=True).then_inc(s_pe)
            tensor.matmul(ps1m1[:], lhsT=w_ap(W1_OFF, 1, 1), rhs=x1T[1], start=False, stop=True).then_inc(s_pe)
            # Prefetch-waits: these DMA semaphores land while PE sits idle
            # waiting for the relus, so waiting here (instead of adjacent to
            # the dependent matmuls) takes the ~110ns/wait sequencer cost
            # off the critical path.
            tensor.wait_ge(dw2k1, 16)
            tensor.wait_ge(dw2, 16)
            # L2: k0 needs h1m0 (DVE), k1 needs h1m1 (ACT)
            tensor.wait_ge(s_dve, 1)
            tensor.matmul(ps2m0[:], lhsT=w_ap(W2_OFF, 0, 0), rhs=h1m0[:], start=True, stop=False)
            tensor.matmul(ps2m1[:], lhsT=w_ap(W2_OFF, 0, 1), rhs=h1m0[:], start=True, stop=False)
            tensor.wait_ge(s_act, 1)
            tensor.matmul(ps2m0[:], lhsT=w2k1[:, 0:128], rhs=h1m1[:], start=False, stop=True).then_inc(s_pe)
            tensor.matmul(ps2m1[:], lhsT=w2k1[:, 128:256], rhs=h1m1[:], start=False, stop=True).then_inc(s_pe)
            # prefetch-waits (all land well before relu2 finishes)
            tensor.wait_ge(dw3, 16)
            tensor.wait_ge(dw3k1, 16)
            tensor.wait_ge(dbrow, 16)
            # The b3 bias matmuls open both L3 psum banks here, in the PE
            # idle window while the relu2 pair runs (the ps2 groups are
            # closed by now, so at most two accumulation groups are open).
            tensor.matmul(ps3m0[:], lhsT=b3m0_row, rhs=ones_row,
                          start=True, stop=False)
            tensor.matmul(ps3m1[:], lhsT=b3m1_row, rhs=ones_row,
                          start=True, stop=False)
            # L3: the four k-matmuls accumulate; m0 (mu) stops first, m1
            # (logvar) right behind.
            tensor.wait_ge(s_dve, 2)
            tensor.matmul(ps3m0[:], lhsT=w_ap(W3_OFF, 0, 0), rhs=h2m0[:], start=False, stop=False)
            tensor.wait_ge(s_act, 2)
            tensor.matmul(ps3m0[:], lhsT=w3k1[:, 0:128], rhs=h2m1[:], start=False, stop=True).then_inc(s_pe)
            tensor.matmul(ps3m1[:], lhsT=w_ap(W3_OFF, 0, 1), rhs=h2m0[:], start=False, stop=False)
            tensor.matmul(ps3m1[:], lhsT=w3k1[:, 128:256], rhs=h2m1[:], start=False, stop=True).then_inc(s_pe)

        @block.vector
        def _(vector):
            # relu m0 halves: (ps + bias) then max 0, one DVE op each
            vector.wait_ge(s_pe, 1)
            vector.tensor_scalar(
                out=h1m0[:], in0=ps1m0[:], scalar1=b_col(0, 0), scalar2=0.0,
                op0=ALU.add, op1=ALU.max,
            ).then_inc(s_dve)
            vector.wait_ge(s_pe, 3)
            vector.tensor_scalar(
                out=h2m0[:], in0=ps2m0[:], scalar1=b_col(1, 0), scalar2=0.0,
                op0=ALU.add, op1=ALU.max,
            ).then_inc(s_dve)
            # prefetch-wait: dab lands long before the tail needs A|B
            vector.wait_ge(dab, 16)
            # tail: c2 = iv*B as soon as exp lands, tmp = mu*A once the mu
            # tanh lands, then c1 = iv*tmp; accumulators give the
            # per-partition sums.
            vector.wait_ge(s_act, 4)
            vector.scalar_tensor_tensor(
                out=scr[:], in0=iv[:], scalar=1.0, in1=B_ap,
                op0=ALU.bypass, op1=ALU.mult, accum_out=out_sb[:, 1:2],
            )
            vector.wait_ge(s_act, 5)
            vector.scalar_tensor_tensor(
                out=tmp[:], in0=mu_ap, scalar=1.0, in1=A_ap,
                op0=ALU.bypass, op1=ALU.mult,
            )
            vector.scalar_tensor_tensor(
                out=scr[:], in0=iv[:], scalar=1.0, in1=tmp[:],
                op0=ALU.bypass, op1=ALU.mult, accum_out=out_sb[:, 0:1],
            ).then_inc(s_dve)

    _split_multi_waits(nc)
    return nc


def _split_multi_waits(nc):
    """This walrus build encodes at most one sync-wait per instruction.
    Hoist extra waits onto same-engine NoOps immediately preceding the
    instruction (engines execute their stream in order, so this is
    semantically identical)."""
    for fn in nc.m.functions:
        for bb in fn.blocks:
            new_insts = []
            for ins in bb.instructions:
                si = ins.sync_info
                if si is not None and len(si.on_wait) > 1:
                    waits = list(si.on_wait)
                    for j, w in enumerate(waits[:-1]):
                        nop = mybir.InstNoOp(
                            name=f"{ins.name}-sw{j}",
                            sync_info=mybir.SyncInfo(on_wait=[w], on_update=[]),
                            bass_nofuse=True,
                            engine=ins.engine,
                        )
                        new_insts.append(nop)
                    si.on_wait = [waits[-1]]
                new_insts.append(ins)
            if len(new_insts) != len(bb.instructions):
                bb.instructions[:] = new_insts


def _pack_inputs(x1, x2, W1, b1, W2, b2, W3, b3):
    f32, f16 = np.float32, np.float16

    def wsec(W):
        W = np.ascontiguousarray(W, f32)
        s = np.empty((P, 512), f16)
        for k in range(2):
            for m in range(2):
                s[:, k * 256 + m * 128 : k * 256 + (m + 1) * 128] = W[
                    k * 128 : (k + 1) * 128, m * 128 : (m + 1) * 128
                ].astype(f16)
        return s

    w1s, w2s, w3s = wsec(W1), wsec(W2), wsec(W3)
    b1 = np.asarray(b1, f32)
    b2 = np.asarray(b2, f32)
    b3 = np.asarray(b3, f32)
    brow = np.zeros((1, 384), f16)
    brow[0, 0:128] = b3[128:256].astype(f16)   # logvar half -> ps3m0
    brow[0, 128:256] = b3[0:128].astype(f16)   # mu half -> ps3m1
    brow[0, 256:384] = 1.0
    x2f = np.asarray(x2, np.float64)
    m1 = x2f.mean(0)
    m2 = (x2f * x2f).mean(0)
    in_maps = []
    for c in range(NCORES):
        bw = np.empty((P, BW_W), f16)
        x1s = np.asarray(x1[c * ROWS : (c + 1) * ROWS], f32)
        x2s = np.asarray(x2[c * ROWS : (c + 1) * ROWS], np.float64)
        bw_u16 = bw.view(np.uint16)
        for li, b in enumerate((b1, b2)):
            for m in range(2):
                c = (2 * li + m) * 2
                bw_u16[:, c : c + 2] = (
                    b[m * 128 : (m + 1) * 128].astype(f32).view(np.uint16).reshape(P, 2)
                )
        bw_u16[:, 8:10] = 0
        bw_u16[:, 10:16] = 0
        bw[:, X1_OFF : X1_OFF + 128] = x1s[:, 0:128].T.astype(f16)
        bw[:, X1_OFF + 128 : X1_OFF + 256] = x1s[:, 128:256].T.astype(f16)
        bw[:, W1_OFF:W2_OFF] = w1s
        bw[:, W2_OFF:W3_OFF] = w2s[:, 0:256]
        w3k0_sw = np.concatenate(
            [w3s[:, 128:256], w3s[:, 0:128]], axis=1)      # [lv|mu] k0
        bw[:, W3_OFF:BW_W] = w3k0_sw
        w2k1a = np.ascontiguousarray(w2s[:, 256:512])
        w3k1a = np.concatenate(
            [w3s[:, 384:512], w3s[:, 256:384]], axis=1)    # [lv|mu] k1
        abb = np.empty((P, 256), f32)
        abb[:, 0:ROWS] = (x2s - m1).T.astype(f32)
        abb[:, ROWS : 2 * ROWS] = (0.5 * (x2s * x2s - m2)).T.astype(f32)
        in_maps.append(
            {"blobW": bw, "blobW2K1": w2k1a, "blobW3K1": w3k1a,
             "brow": brow, "blobAB": abb}
        )
    return in_maps


def _run(in_maps, **kwargs):
    global _module_cache
    if _module_cache is None:
        _module_cache = _build_module()
    return run_bass_kernel_spmd(
        _module_cache, in_maps, core_ids=list(range(NCORES)), **kwargs
    )


def _combine(results):
    tot = 0.0
    for r in results:
        o = np.asarray(r["out"], np.float64)
        tot += float(np.sum(o[:, 0] - o[:, 1]))
    return np.float32(tot / N)


def kernel(x1, x2, W1, b1, W2, b2, W3, b3):
    in_maps = _pack_inputs(x1, x2, W1, b1, W2, b2, W3, b3)
    res = _run(in_maps)
    return _combine(res.results)


# revision 63
# speedup vs baseline: 1.0238x; 1.0238x over previous
"""CLUB mutual-information upper bound (loss_fn) on 8 Trainium2 NeuronCores.

Math: the reference computes
    h  = relu(x1 @ W1 + b1); h = relu(h @ W2 + b2); g = tanh(h @ W3 + b3)
    mu, logvar = split(g); iv = exp(-logvar)
    pos = -0.5 (mu - x2)^2 iv
    neg = -0.5 mean_j[(mu_i - x2_j)^2] iv     (the [N,N,D] pairwise term)
    mi  = mean_i sum_d (pos - neg)
With m1 = mean_j x2, m2 = mean_j x2^2 (host-computed, global over all N),
the pairwise term collapses:  pos - neg = iv (mu*A - B) where A = x2 - m1,
B = 0.5 (x2^2 - m2). Each of the 8 cores runs its 128-row shard through the
MLP and accumulates c1_d = sum_i iv*mu*A, c2_d = sum_i iv*B per feature
partition; the host finishes mi = sum_cores sum_d (c1 - c2) / N.

Measured-window model (from neuron-profile traces): exec_time =
(body barrier end - first framework MEMSET) + ~7.39us fixed tail (the
postamble resets ~250 hw semaphores serially before the completion notify).
Only the body is optimizable; this version's body is ~8.5-8.8us.

Structure (each element validated against trace anchors):
  - One fp16 HWDGE stream on the SP ring: [bias|x1T|W1k0] (dw1a, gates the
    first matmuls) -> [W1k1] (dw1b) -> [W2k0] -> [W3k0]. The ring streams
    only ~140-165GB/s, so the k1 halves of W2/W3 ride the parallel SWDGE
    path (w2k1 -> brow -> w3k1 -> A|B). DMA completion = data end + ~900ns
    sem-prop, plus a ~45ns/inc 16-increment train serialized per queue.
  - Biases: b1/b2 as f32 bit-packed fp16 column pairs applied via the ACT
    relu bias operand (m1 halves) and DVE tensor_scalar scalar-AP add (m0
    halves); b3 via outer-product matmuls (1-partition rows x ones) that
    open the two L3 psum banks during the relu2 idle window.
  - Layer matmuls split k(contraction)/m(feature-half); relu m0 on DVE and
    m1 on ACT run in parallel; per-layer rhs comes from the previous
    layer's two relu outputs, so k0 matmuls start as soon as the DVE half
    lands.
  - L3: logvar accumulates in ps3m0 and closes one matmul earlier than mu
    (k-matmul order k0lv, k1lv(close), k0mu, k1mu), so the serial scalar
    chain tanh(lv) -> exp starts ~200ns sooner; tanh(mu) then reads ps3m1
    without a semaphore wait (it retires <=220ns after ps3m0 closes while
    tanh+exp occupy scalar >=650ns).
  - Tail on DVE: c2 = iv*B (at exp), tmp = mu*A (at mu-tanh), c1 = iv*tmp,
    with per-partition accumulators into out[128,2].
  - The out DMA is released on the exp semaphore: its ~640ns issue plus the
    idle-queue first-read latency covers the trailing writes, and the fixed
    postamble covers the flight. Warmup dummy matmuls keep the PE busy
    until dw1a lands.

Known-negative variants (all measured): x1 or W-halves first on SWDGE (the
Pool engine issues ~650ns late), dual-ring HWDGE streaming (rings share
bandwidth), merged single-bank L3 psum (group serialization), fp16 A/B
(costs 7e-3 accuracy), earlier out-DMA release (races the accum writes).
"""
import sys
from contextlib import ExitStack

import numpy as np

sys.path.insert(0, "/opt/trn_rl_repo")

import concourse.bass as bass
from concourse import mybir
from concourse.bass_utils import run_bass_kernel_spmd

F32 = mybir.dt.float32
F16 = mybir.dt.float16
NCORES = 8
N = 1024
X1D = 256
X2D = 128
HID = 256
ROWS = N // NCORES  # 128
P = 128

# blobW (fp16) [128, 1808]:
#   [0:16)       bias cols: f32 values bit-packed as fp16 pairs;
#                pair (2l+m)*2 = b_{l+1}[m*128:(m+1)*128]; pair 4 = zeros
#                (AP biases everywhere keep the framework from emitting
#                const-AP memsets in the preamble)
#   [16:272)     x1T   col 16 + k*128 + j = x1s[j, k*128+p]
#   [272:784)    W1    col 272 + k*256 + m*128 + j = W1[k*128+p, m*128+j]
#                (k-major so [bias|x1|W1k0] is one contiguous DMA -> dw1a,
#                 W1k1 a second -> dw1b: k0 matmuls start one DMA earlier)
#   [784:1040)   W2 k0 only
#   [1040:1296)  W3 k0 only — the k1 halves of W2 and W3 ride the SWDGE
#                path: the HWDGE ring streams only ~140-165GB/s, so
#                shedding 132KB pulls dw2 in before relu1 finishes
# blobW2K1/blobW3K1 (fp16) [128, 256]: k1 halves, col m*128+j = W[128+p, m*128+j]
# brow (fp16) [1, 384]: [0:128) b3m0, [128:256) b3m1, [256:384) ones
# blobAB (fp32) [128, 256]: [0:128) A = (x2s - m1).T ; [128:256) B = 0.5(x2s^2 - m2).T
#   (f32: the final value is a small difference of large sums, fp16 A/B cost
#    ~7e-3 of relative error on the result)
BIAS_OFF = 0
X1_OFF = 16
W1_OFF = 272
W1K1_OFF = 528
W2_OFF = 784
W3_OFF = 1040
BW_W = 1296

N_DUMMIES = 18
N_NARROW = 4

_module_cache = None


def _build_module(n_dummies=N_DUMMIES, n_narrow=N_NARROW):
    nc = bass.Bass()
    blobW = nc.declare_dram_parameter("blobW", [P, BW_W], F16, isOutput=False)
    blobW2K1 = nc.declare_dram_parameter("blobW2K1", [P, 256], F16, isOutput=False)
    blobW3K1 = nc.declare_dram_parameter("blobW3K1", [P, 256], F16, isOutput=False)
    brow = nc.declare_dram_parameter("brow", [1, 384], F16, isOutput=False)
    blobAB = nc.declare_dram_parameter("blobAB", [P, 256], F32, isOutput=False)
    out = nc.declare_dram_parameter("out", [P, 2], F32, isOutput=True)

    AF = mybir.ActivationFunctionType
    ALU = mybir.AluOpType

    with ExitStack() as ctx:
        ec = ctx.enter_context
        bw = ec(nc.sbuf_tensor("bw", [P, BW_W], F16))
        w2k1 = ec(nc.sbuf_tensor("w2k1", [P, 256], F16))
        w3k1 = ec(nc.sbuf_tensor("w3k1", [P, 256], F16))
        brow_sb = ec(nc.sbuf_tensor("brow_sb", [1, 384], F16))
        ab = ec(nc.sbuf_tensor("ab", [P, 256], F32))
        h1m0 = ec(nc.sbuf_tensor("h1m0", [P, ROWS], F16))
        h1m1 = ec(nc.sbuf_tensor("h1m1", [P, ROWS], F16))
        h2m0 = ec(nc.sbuf_tensor("h2m0", [P, ROWS], F16))
        h2m1 = ec(nc.sbuf_tensor("h2m1", [P, ROWS], F16))
        g = ec(nc.sbuf_tensor("g", [P, 2 * ROWS], F32))  # mu | logvar->tanh
        iv = ec(nc.sbuf_tensor("iv", [P, ROWS], F32))
        tmp = ec(nc.sbuf_tensor("tmp", [P, ROWS], F32))
        scr = ec(nc.sbuf_tensor("scr", [P, ROWS], F32))
        out_sb = ec(nc.sbuf_tensor("out_sb", [P, 2], F32))
        ps1m0 = ec(nc.psum_tensor("ps1m0", [P, ROWS], F32))
        ps1m1 = ec(nc.psum_tensor("ps1m1", [P, ROWS], F32))
        ps2m0 = ec(nc.psum_tensor("ps2m0", [P, ROWS], F32))
        ps2m1 = ec(nc.psum_tensor("ps2m1", [P, ROWS], F32))
        ps3m0 = ec(nc.psum_tensor("ps3m0", [P, ROWS], F32))
        ps3m1 = ec(nc.psum_tensor("ps3m1", [P, ROWS], F32))
        psw = ec(nc.psum_tensor("psw", [P, ROWS], F32))
        dbrow = ec(nc.semaphore("dbrow"))
        dw1a = ec(nc.semaphore("dw1a"))
        dw1b = ec(nc.semaphore("dw1b"))
        dw2 = ec(nc.semaphore("dw2"))
        dw3 = ec(nc.semaphore("dw3"))
        dw2k1 = ec(nc.semaphore("dw2k1"))
        dw3k1 = ec(nc.semaphore("dw3k1"))
        dab = ec(nc.semaphore("dab"))
        s_pe = ec(nc.semaphore("s_pe"))
        s_act = ec(nc.semaphore("s_act"))
        s_dve = ec(nc.semaphore("s_dve"))
        dout = ec(nc.semaphore("dout"))
        block = ec(nc.Block())

        # m-major W1 split: dw1a carries the whole m0 path (x1 both k
        # chunks + W1 m0 halves) so ps1m0 closes one DMA chunk earlier;
        # dw1b carries only the W1 m1 halves.
        x1T = [bw[:, 16:144], bw[:, 272:400]]
        w1_k0m0 = bw[:, 144:272]
        w1_k1m0 = bw[:, 400:528]
        w1_k0m1 = bw[:, 528:656]
        w1_k1m1 = bw[:, 656:784]
        zcol = bw[:, 8:10].bitcast(F32)  # zero bias pair
        A_ap = ab[:, 0:ROWS]
        B_ap = ab[:, ROWS : 2 * ROWS]

        def w_ap(woff, k, m):
            c = woff + k * 256 + m * 128
            return bw[:, c : c + 128]

        def b_col(l, m):
            c = BIAS_OFF + (2 * l + m) * 2
            return bw[:, c : c + 2].bitcast(F32)

        b3m0_row = brow_sb[0:1, 0:128]
        b3m1_row = brow_sb[0:1, 128:256]
        ones_row = brow_sb[0:1, 256:384]

        mu_ap = g[:, 0:ROWS]
        lv_ap = g[:, ROWS : 2 * ROWS]

        @block.sync
        def _(sync):
            # HWDGE stream: the L1-critical prefix first ([bias|x1|W1k0] then
            # [W1k1] so k0 matmuls start one DMA-chunk earlier), then W2, W3
            # pipelining behind on the same ring.
            sync.dma_start(
                out=bw[:, 0:W1K1_OFF], in_=blobW[:, 0:W1K1_OFF]
            ).then_inc(dw1a, 16)
            sync.dma_start(
                out=bw[:, W1K1_OFF:W2_OFF], in_=blobW[:, W1K1_OFF:W2_OFF]
            ).then_inc(dw1b, 16)
            sync.dma_start(
                out=bw[:, W2_OFF:W3_OFF], in_=blobW[:, W2_OFF:W3_OFF]
            ).then_inc(dw2, 16)
            sync.dma_start(
                out=bw[:, W3_OFF:BW_W], in_=blobW[:, W3_OFF:BW_W]
            ).then_inc(dw3, 16)  # W3 k0 only
            # out DMA released when exp is done (s_act>=4): the ~640ns
            # issue + the idle-queue first-read latency (~770ns) covers the
            # trailing mu-tanh + DVE accum writes with ~600ns margin, and
            # the fixed end-of-kernel semaphore sweep covers the flight.
            sync.wait_ge(s_act, 4)
            sync.dma_start(out=out[:], in_=out_sb[:]).then_inc(dout, 16)

        @block.gpsimd
        def _(gpsimd):
            # SWDGE has its own descriptor generator and runs in parallel
            # with the HWDGE ring: W3's k1 half first (sheds 66KB off the
            # HWDGE stream), then the tiny b3/ones row (needed by the L3
            # bias matmuls mid-chain), then the f32 A|B (tail only).
            # (x1 via SWDGE was tried and lost ~700ns: the Pool engine
            # issues its first DMA ~650ns later than Sync does.)
            gpsimd.dma_start(out=w2k1[:], in_=blobW2K1[:]).then_inc(dw2k1, 16)
            gpsimd.dma_start(out=brow_sb[0:1, :], in_=brow[0:1, :]).then_inc(dbrow, 16)
            gpsimd.dma_start(out=w3k1[:], in_=blobW3K1[:]).then_inc(dw3k1, 16)
            gpsimd.dma_start(out=ab[:], in_=blobAB[:]).then_inc(dab, 16)

        @block.scalar
        def _(scalar):
            # dummy activations: ACT table load (relu/tanh/exp) starts early.
            # All biases are APs (zcol) so the framework emits no const-AP
            # memsets in the preamble (zcol holds garbage this early; the
            # dummy outputs are dead stores into scr).
            scalar.activation(
                out=scr[0:1, 0:1], in_=scr[0:1, 0:1], func=AF.Relu,
                bias=zcol[0:1, 0:1], scale=1.0
            )
            scalar.activation(
                out=scr[0:1, 0:1], in_=scr[0:1, 0:1], func=AF.Tanh,
                bias=zcol[0:1, 0:1], scale=1.0
            )
            scalar.activation(
                out=scr[0:1, 0:1], in_=scr[0:1, 0:1], func=AF.Exp,
                bias=zcol[0:1, 0:1], scale=0.0
            )
            # relu m1 halves (bias via ACT bias operand, fp16 [128,1] AP)
            scalar.wait_ge(s_pe, 2)
            scalar.activation(
                out=h1m1[:], in_=ps1m1[:], func=AF.Relu, bias=b_col(0, 1), scale=1.0
            ).then_inc(s_act)
            scalar.wait_ge(s_pe, 4)
            scalar.activation(
                out=h2m1[:], in_=ps2m1[:], func=AF.Relu, bias=b_col(1, 1), scale=1.0
            ).then_inc(s_act)
            # logvar lives in ps3m0, which closes one matmul earlier than
            # ps3m1 -> the tanh->exp chain starts ~107ns sooner. The mu tanh
            # reads ps3m1 without a wait: k1m1 retires <=110ns after k1m0
            # while tanh_lv+exp occupy scalar >=650ns.
            scalar.wait_ge(s_pe, 5)
            scalar.activation(
                out=lv_ap, in_=ps3m0[:], func=AF.Tanh, bias=zcol, scale=1.0
            ).then_inc(s_act)
            scalar.activation(
                out=iv[:], in_=lv_ap, func=AF.Exp, bias=zcol, scale=-1.0
            ).then_inc(s_act)
            scalar.activation(
                out=mu_ap, in_=ps3m1[:], func=AF.Tanh, bias=zcol, scale=1.0
            ).then_inc(s_act)

        @block.tensor
        def _(tensor):
            # warmup: keep the PE clock ramped while the input DMA flies.
            for _i in range(n_dummies):
                tensor.matmul(psw[:], lhsT=bw[:, 0:128], rhs=bw[:, 0:128],
                              start=True, stop=True)
            for _i in range(n_narrow):
                tensor.matmul(psw[:, 0:32], lhsT=bw[:, 0:128],
                              rhs=bw[:, 0:32], start=True, stop=True)
            # L1: bias comes via the relu, so psum groups start at k0.
            # [bias|x1|W1k0] arrive one HWDGE chunk before W1k1.
            tensor.wait_ge(dw1a, 16)
            tensor.matmul(ps1m0[:], lhsT=w1_k0m0, rhs=x1T[0], start=True, stop=False)
            tensor.matmul(ps1m0[:], lhsT=w1_k1m0, rhs=x1T[1], start=False, stop=True).then_inc(s_pe)
            tensor.wait_ge(dw1b, 16)
            tensor.matmul(ps1m1[:], lhsT=w1_k0m1, rhs=x1T[0], start=True, stop=False)
            tensor.matmul(ps1m1[:], lhsT=w1_k1m1, rhs=x1T[1], start=False, stop=True).then_inc(s_pe)
            # Prefetch-waits: these DMA semaphores land while PE sits idle
            # waiting for the relus, so waiting here (instead of adjacent to
            # the dependent matmuls) takes the ~110ns/wait sequencer cost
            # off the critical path.
            tensor.wait_ge(dw2k1, 16)
            tensor.wait_ge(dw2, 16)
            # L2: k0 needs h1m0 (DVE), k1 needs h1m1 (ACT)
            tensor.wait_ge(s_dve, 1)
            tensor.matmul(ps2m0[:], lhsT=w_ap(W2_OFF, 0, 0), rhs=h1m0[:], start=True, stop=False)
            tensor.matmul(ps2m1[:], lhsT=w_ap(W2_OFF, 0, 1), rhs=h1m0[:], start=True, stop=False)
            tensor.wait_ge(s_act, 1)
            tensor.matmul(ps2m0[:], lhsT=w2k1[:, 0:128], rhs=h1m1[:], start=False, stop=True).then_inc(s_pe)
            tensor.matmul(ps2m1[:], lhsT=w2k1[:, 128:256], rhs=h1m1[:], start=False, stop=True).then_inc(s_pe)
            # prefetch-waits (all land well before relu2 finishes)
            tensor.wait_ge(dw3, 16)
            tensor.wait_ge(dw3k1, 16)
            tensor.wait_ge(dbrow, 16)
            # The b3 bias matmuls open both L3 psum banks here, in the PE
            # idle window while the relu2 pair runs (the ps2 groups are
            # closed by now, so at most two accumulation groups are open).
            tensor.matmul(ps3m0[:], lhsT=b3m0_row, rhs=ones_row,
                          start=True, stop=False)
            tensor.matmul(ps3m1[:], lhsT=b3m1_row, rhs=ones_row,
                          start=True, stop=False)
            # L3: the four k-matmuls accumulate; m0 (mu) stops first, m1
            # (logvar) right behind.
            tensor.wait_ge(s_dve, 2)
            tensor.matmul(ps3m0[:], lhsT=w_ap(W3_OFF, 0, 0), rhs=h2m0[:], start=False, stop=False)
            tensor.wait_ge(s_act, 2)
            tensor.matmul(ps3m0[:], lhsT=w3k1[:, 0:128], rhs=h2m1[:], start=False, stop=True).then_inc(s_pe)
            tensor.matmul(ps3m1[:], lhsT=w_ap(W3_OFF, 0, 1), rhs=h2m0[:], start=False, stop=False)
            tensor.matmul(ps3m1[:], lhsT=w3k1[:, 128:256], rhs=h2m1[:], start=False, stop=True).then_inc(s_pe)

        @block.vector
        def _(vector):
            # relu m0 halves: (ps + bias) then max 0, one DVE op each
            vector.wait_ge(s_pe, 1)
            vector.tensor_scalar(
                out=h1m0[:], in0=ps1m0[:], scalar1=b_col(0, 0), scalar2=0.0,
                op0=ALU.add, op1=ALU.max,
            ).then_inc(s_dve)
            vector.wait_ge(s_pe, 3)
            vector.tensor_scalar(
                out=h2m0[:], in0=ps2m0[:], scalar1=b_col(1, 0), scalar2=0.0,
                op0=ALU.add, op1=ALU.max,
            ).then_inc(s_dve)
            # prefetch-wait: dab lands long before the tail needs A|B
            vector.wait_ge(dab, 16)
            # tail: c2 = iv*B as soon as exp lands, tmp = mu*A once the mu
            # tanh lands, then c1 = iv*tmp; accumulators give the
            # per-partition sums.
            vector.wait_ge(s_act, 4)
            vector.scalar_tensor_tensor(
                out=scr[:], in0=iv[:], scalar=1.0, in1=B_ap,
                op0=ALU.bypass, op1=ALU.mult, accum_out=out_sb[:, 1:2],
            )
            vector.wait_ge(s_act, 5)
            vector.scalar_tensor_tensor(
                out=tmp[:], in0=mu_ap, scalar=1.0, in1=A_ap,
                op0=ALU.bypass, op1=ALU.mult,
            )
            vector.scalar_tensor_tensor(
                out=scr[:], in0=iv[:], scalar=1.0, in1=tmp[:],
                op0=ALU.bypass, op1=ALU.mult, accum_out=out_sb[:, 0:1],
            ).then_inc(s_dve)

    _split_multi_waits(nc)
    return nc


def _split_multi_waits(nc):
    """This walrus build encodes at most one sync-wait per instruction.
    Hoist extra waits onto same-engine NoOps immediately preceding the
    instruction (engines execute their stream in order, so this is
    semantically identical)."""
    for fn in nc.m.functions:
        for bb in fn.blocks:
            new_insts = []
            for ins in bb.instructions:
                si = ins.sync_info
                if si is not None and len(si.on_wait) > 1:
                    waits = list(si.on_wait)
                    for j, w in enumerate(waits[:-1]):
                        nop = mybir.InstNoOp(
                            name=f"{ins.name}-sw{j}",
                            sync_info=mybir.SyncInfo(on_wait=[w], on_update=[]),
                            bass_nofuse=True,
                            engine=ins.engine,
                        )
                        new_insts.append(nop)
                    si.on_wait = [waits[-1]]
                new_insts.append(ins)
            if len(new_insts) != len(bb.instructions):
                bb.instructions[:] = new_insts


def _pack_inputs(x1, x2, W1, b1, W2, b2, W3, b3):
    f32, f16 = np.float32, np.float16

    def wsec(W):
        W = np.ascontiguousarray(W, f32)
        s = np.empty((P, 512), f16)
        for k in range(2):
            for m in range(2):
                s[:, k * 256 + m * 128 : k * 256 + (m + 1) * 128] = W[
                    k * 128 : (k + 1) * 128, m * 128 : (m + 1) * 128
                ].astype(f16)
        return s

    w1s, w2s, w3s = wsec(W1), wsec(W2), wsec(W3)
    b1 = np.asarray(b1, f32)
    b2 = np.asarray(b2, f32)
    b3 = np.asarray(b3, f32)
    brow = np.zeros((1, 384), f16)
    brow[0, 0:128] = b3[128:256].astype(f16)   # logvar half -> ps3m0
    brow[0, 128:256] = b3[0:128].astype(f16)   # mu half -> ps3m1
    brow[0, 256:384] = 1.0
    x2f = np.asarray(x2, np.float64)
    m1 = x2f.mean(0)
    m2 = (x2f * x2f).mean(0)
    in_maps = []
    for c in range(NCORES):
        bw = np.empty((P, BW_W), f16)
        x1s = np.asarray(x1[c * ROWS : (c + 1) * ROWS], f32)
        x2s = np.asarray(x2[c * ROWS : (c + 1) * ROWS], np.float64)
        bw_u16 = bw.view(np.uint16)
        for li, b in enumerate((b1, b2)):
            for m in range(2):
                c = (2 * li + m) * 2
                bw_u16[:, c : c + 2] = (
                    b[m * 128 : (m + 1) * 128].astype(f32).view(np.uint16).reshape(P, 2)
                )
        bw_u16[:, 8:10] = 0
        bw_u16[:, 10:16] = 0
        bw[:, 16:144] = x1s[:, 0:128].T.astype(f16)
        bw[:, 144:272] = w1s[:, 0:128]      # W1 k0 m0
        bw[:, 272:400] = x1s[:, 128:256].T.astype(f16)
        bw[:, 400:528] = w1s[:, 256:384]    # W1 k1 m0
        bw[:, 528:656] = w1s[:, 128:256]    # W1 k0 m1
        bw[:, 656:784] = w1s[:, 384:512]    # W1 k1 m1
        bw[:, W2_OFF:W3_OFF] = w2s[:, 0:256]
        w3k0_sw = np.concatenate(
            [w3s[:, 128:256], w3s[:, 0:128]], axis=1)      # [lv|mu] k0
        bw[:, W3_OFF:BW_W] = w3k0_sw
        w2k1a = np.ascontiguousarray(w2s[:, 256:512])
        w3k1a = np.concatenate(
            [w3s[:, 384:512], w3s[:, 256:384]], axis=1)    # [lv|mu] k1
        abb = np.empty((P, 256), f32)
        abb[:, 0:ROWS] = (x2s - m1).T.astype(f32)
        abb[:, ROWS : 2 * ROWS] = (0.5 * (x2s * x2s - m2)).T.astype(f32)
        in_maps.append(
            {"blobW": bw, "blobW2K1": w2k1a, "blobW3K1": w3k1a,
             "brow": brow, "blobAB": abb}
        )
    return in_maps


def _run(in_maps, **kwargs):
    global _module_cache
    if _module_cache is None:
        _module_cache = _build_module()
    return run_bass_kernel_spmd(
        _module_cache, in_maps, core_ids=list(range(NCORES)), **kwargs
    )


def _combine(results):
    tot = 0.0
    for r in results:
        o = np.asarray(r["out"], np.float64)
        tot += float(np.sum(o[:, 0] - o[:, 1]))
    return np.float32(tot / N)


def kernel(x1, x2, W1, b1, W2, b2, W3, b3):
    in_maps = _pack_inputs(x1, x2, W1, b1, W2, b2, W3, b3)
    res = _run(in_maps)
    return _combine(res.results)


# revision 64
# speedup vs baseline: 1.0517x; 1.0272x over previous
"""CLUB mutual-information upper bound (loss_fn) on 8 Trainium2 NeuronCores.

Math: the reference computes
    h  = relu(x1 @ W1 + b1); h = relu(h @ W2 + b2); g = tanh(h @ W3 + b3)
    mu, logvar = split(g); iv = exp(-logvar)
    pos = -0.5 (mu - x2)^2 iv
    neg = -0.5 mean_j[(mu_i - x2_j)^2] iv     (the [N,N,D] pairwise term)
    mi  = mean_i sum_d (pos - neg)
With m1 = mean_j x2, m2 = mean_j x2^2 (host-computed, global over all N),
the pairwise term collapses:  pos - neg = iv (mu*A - B) where A = x2 - m1,
B = 0.5 (x2^2 - m2). Each of the 8 cores runs its 128-row shard through the
MLP and accumulates c1_d = sum_i iv*mu*A, c2_d = sum_i iv*B per feature
partition; the host finishes mi = sum_cores sum_d (c1 - c2) / N.

Measured-window model (from neuron-profile traces): exec_time =
(body barrier end - first framework MEMSET) + ~7.39us fixed tail (the
postamble resets ~250 hw semaphores serially before the completion notify).
Only the body is optimizable; this version's body is ~8.5-8.8us.

Structure (each element validated against trace anchors):
  - One fp16 HWDGE stream on the SP ring: [bias|x1T|W1k0] (dw1a, gates the
    first matmuls) -> [W1k1] (dw1b) -> [W2k0] -> [W3k0]. The ring streams
    only ~140-165GB/s, so the k1 halves of W2/W3 ride the parallel SWDGE
    path (w2k1 -> brow -> w3k1 -> A|B). DMA completion = data end + ~900ns
    sem-prop, plus a ~45ns/inc 16-increment train serialized per queue.
  - Biases: b1/b2 as f32 bit-packed fp16 column pairs applied via the ACT
    relu bias operand (m1 halves) and DVE tensor_scalar scalar-AP add (m0
    halves); b3 via outer-product matmuls (1-partition rows x ones) that
    open the two L3 psum banks during the relu2 idle window.
  - Layer matmuls split k(contraction)/m(feature-half); relu m0 on DVE and
    m1 on ACT run in parallel; per-layer rhs comes from the previous
    layer's two relu outputs, so k0 matmuls start as soon as the DVE half
    lands.
  - L3: logvar accumulates in ps3m0 and closes one matmul earlier than mu
    (k-matmul order k0lv, k1lv(close), k0mu, k1mu), so the serial scalar
    chain tanh(lv) -> exp starts ~200ns sooner; tanh(mu) then reads ps3m1
    without a semaphore wait (it retires <=220ns after ps3m0 closes while
    tanh+exp occupy scalar >=650ns).
  - Tail on DVE: c2 = iv*B (at exp), tmp = mu*A (at mu-tanh), c1 = iv*tmp,
    with per-partition accumulators into out[128,2].
  - The out DMA is released on the exp semaphore: its ~640ns issue plus the
    idle-queue first-read latency covers the trailing writes, and the fixed
    postamble covers the flight. Warmup dummy matmuls keep the PE busy
    until dw1a lands.

Known-negative variants (all measured): x1 or W-halves first on SWDGE (the
Pool engine issues ~650ns late), dual-ring HWDGE streaming (rings share
bandwidth), merged single-bank L3 psum (group serialization), fp16 A/B
(costs 7e-3 accuracy), earlier out-DMA release (races the accum writes).
"""
import sys
from contextlib import ExitStack

import numpy as np

sys.path.insert(0, "/opt/trn_rl_repo")

import concourse.bass as bass
from concourse import mybir
from concourse.bass_utils import run_bass_kernel_spmd

F32 = mybir.dt.float32
F16 = mybir.dt.float16
NCORES = 8
N = 1024
X1D = 256
X2D = 128
HID = 256
ROWS = N // NCORES  # 128
P = 128

# blobW (fp16) [128, 1808]:
#   [0:16)       bias cols: f32 values bit-packed as fp16 pairs;
#                pair (2l+m)*2 = b_{l+1}[m*128:(m+1)*128]; pair 4 = zeros
#                (AP biases everywhere keep the framework from emitting
#                const-AP memsets in the preamble)
#   [16:272)     x1T   col 16 + k*128 + j = x1s[j, k*128+p]
#   [272:784)    W1    col 272 + k*256 + m*128 + j = W1[k*128+p, m*128+j]
#                (k-major so [bias|x1|W1k0] is one contiguous DMA -> dw1a,
#                 W1k1 a second -> dw1b: k0 matmuls start one DMA earlier)
#   [784:1040)   W2 k0 only
#   [1040:1296)  W3 k0 only — the k1 halves of W2 and W3 ride the SWDGE
#                path: the HWDGE ring streams only ~140-165GB/s, so
#                shedding 132KB pulls dw2 in before relu1 finishes
# blobW2K1/blobW3K1 (fp16) [128, 256]: k1 halves, col m*128+j = W[128+p, m*128+j]
# brow (fp16) [1, 384]: [0:128) b3m0, [128:256) b3m1, [256:384) ones
# blobAB (fp32) [128, 256]: [0:128) A = (x2s - m1).T ; [128:256) B = 0.5(x2s^2 - m2).T
#   (f32: the final value is a small difference of large sums, fp16 A/B cost
#    ~7e-3 of relative error on the result)
BIAS_OFF = 0
X1_OFF = 16
W1_OFF = 272
W1K1_OFF = 528
W2_OFF = 784
W3_OFF = 1040
BW_W = 1296

N_DUMMIES = 18
N_NARROW = 4

_module_cache = None


def _build_module(n_dummies=N_DUMMIES, n_narrow=N_NARROW):
    nc = bass.Bass()
    blobW = nc.declare_dram_parameter("blobW", [P, BW_W], F16, isOutput=False)
    blobW2K1 = nc.declare_dram_parameter("blobW2K1", [P, 256], F16, isOutput=False)
    blobW3K1 = nc.declare_dram_parameter("blobW3K1", [P, 256], F16, isOutput=False)
    brow = nc.declare_dram_parameter("brow", [1, 384], F16, isOutput=False)
    blobAB = nc.declare_dram_parameter("blobAB", [P, 256], F32, isOutput=False)
    out = nc.declare_dram_parameter("out", [P, 2], F32, isOutput=True)

    AF = mybir.ActivationFunctionType
    ALU = mybir.AluOpType

    with ExitStack() as ctx:
        ec = ctx.enter_context
        bw = ec(nc.sbuf_tensor("bw", [P, BW_W], F16))
        w2k1 = ec(nc.sbuf_tensor("w2k1", [P, 256], F16))
        w3k1 = ec(nc.sbuf_tensor("w3k1", [P, 256], F16))
        brow_sb = ec(nc.sbuf_tensor("brow_sb", [1, 384], F16))
        ab = ec(nc.sbuf_tensor("ab", [P, 256], F32))
        h1m0 = ec(nc.sbuf_tensor("h1m0", [P, ROWS], F16))
        h1m1 = ec(nc.sbuf_tensor("h1m1", [P, ROWS], F16))
        h2m0 = ec(nc.sbuf_tensor("h2m0", [P, ROWS], F16))
        h2m1 = ec(nc.sbuf_tensor("h2m1", [P, ROWS], F16))
        g = ec(nc.sbuf_tensor("g", [P, 2 * ROWS], F32))  # mu | logvar->tanh
        iv = ec(nc.sbuf_tensor("iv", [P, ROWS], F32))
        tmp = ec(nc.sbuf_tensor("tmp", [P, ROWS], F32))
        scr = ec(nc.sbuf_tensor("scr", [P, ROWS], F32))
        out_sb = ec(nc.sbuf_tensor("out_sb", [P, 2], F32))
        ps1m0 = ec(nc.psum_tensor("ps1m0", [P, ROWS], F32))
        ps1m1 = ec(nc.psum_tensor("ps1m1", [P, ROWS], F32))
        ps2m0 = ec(nc.psum_tensor("ps2m0", [P, ROWS], F32))
        ps2m1 = ec(nc.psum_tensor("ps2m1", [P, ROWS], F32))
        ps3m0 = ec(nc.psum_tensor("ps3m0", [P, ROWS], F32))
        ps3m1 = ec(nc.psum_tensor("ps3m1", [P, ROWS], F32))
        psw = ec(nc.psum_tensor("psw", [P, ROWS], F32))
        dbrow = ec(nc.semaphore("dbrow"))
        dw1a = ec(nc.semaphore("dw1a"))
        dw2 = ec(nc.semaphore("dw2"))
        dw3 = ec(nc.semaphore("dw3"))
        dw2k1 = ec(nc.semaphore("dw2k1"))
        dw3k1 = ec(nc.semaphore("dw3k1"))
        dab = ec(nc.semaphore("dab"))
        s_pe = ec(nc.semaphore("s_pe"))
        s_act = ec(nc.semaphore("s_act"))
        s_dve = ec(nc.semaphore("s_dve"))
        dout = ec(nc.semaphore("dout"))
        block = ec(nc.Block())

        # m-major W1 split: dw1a carries the whole m0 path (x1 both k
        # chunks + W1 m0 halves) so ps1m0 closes one DMA chunk earlier;
        # dw1b carries only the W1 m1 halves.
        x1T = [bw[:, 16:144], bw[:, 272:400]]
        w1_k0m0 = bw[:, 144:272]
        w1_k1m0 = bw[:, 400:528]
        w1_k0m1 = bw[:, 528:656]
        w1_k1m1 = bw[:, 656:784]
        zcol = bw[:, 8:10].bitcast(F32)  # zero bias pair
        A_ap = ab[:, 0:ROWS]
        B_ap = ab[:, ROWS : 2 * ROWS]

        def w_ap(woff, k, m):
            c = woff + k * 256 + m * 128
            return bw[:, c : c + 128]

        def b_col(l, m):
            c = BIAS_OFF + (2 * l + m) * 2
            return bw[:, c : c + 2].bitcast(F32)

        b3m0_row = brow_sb[0:1, 0:128]
        b3m1_row = brow_sb[0:1, 128:256]
        ones_row = brow_sb[0:1, 256:384]

        mu_ap = g[:, 0:ROWS]
        lv_ap = g[:, ROWS : 2 * ROWS]

        @block.sync
        def _(sync):
            # HWDGE stream: the L1-critical prefix first ([bias|x1|W1k0] then
            # [W1k1] so k0 matmuls start one DMA-chunk earlier), then W2, W3
            # pipelining behind on the same ring.
            # single DMA for bias+x1+all of W1: a second chunk's 16-inc
            # semaphore train (serialized ~400-700ns behind the first's)
            # costs more than the merged DMA's extra 66KB of data time
            sync.dma_start(
                out=bw[:, 0:W2_OFF], in_=blobW[:, 0:W2_OFF]
            ).then_inc(dw1a, 16)
            sync.dma_start(
                out=bw[:, W2_OFF:W3_OFF], in_=blobW[:, W2_OFF:W3_OFF]
            ).then_inc(dw2, 16)
            sync.dma_start(
                out=bw[:, W3_OFF:BW_W], in_=blobW[:, W3_OFF:BW_W]
            ).then_inc(dw3, 16)  # W3 k0 only
            # out DMA released when exp is done (s_act>=4): the ~640ns
            # issue + the idle-queue first-read latency (~770ns) covers the
            # trailing mu-tanh + DVE accum writes with ~600ns margin, and
            # the fixed end-of-kernel semaphore sweep covers the flight.
            sync.wait_ge(s_act, 4)
            sync.dma_start(out=out[:], in_=out_sb[:]).then_inc(dout, 16)

        @block.gpsimd
        def _(gpsimd):
            # SWDGE has its own descriptor generator and runs in parallel
            # with the HWDGE ring: W3's k1 half first (sheds 66KB off the
            # HWDGE stream), then the tiny b3/ones row (needed by the L3
            # bias matmuls mid-chain), then the f32 A|B (tail only).
            # (x1 via SWDGE was tried and lost ~700ns: the Pool engine
            # issues its first DMA ~650ns later than Sync does.)
            gpsimd.dma_start(out=w2k1[:], in_=blobW2K1[:]).then_inc(dw2k1, 16)
            gpsimd.dma_start(out=brow_sb[0:1, :], in_=brow[0:1, :]).then_inc(dbrow, 16)
            gpsimd.dma_start(out=w3k1[:], in_=blobW3K1[:]).then_inc(dw3k1, 16)
            gpsimd.dma_start(out=ab[:], in_=blobAB[:]).then_inc(dab, 16)

        @block.scalar
        def _(scalar):
            # dummy activations: ACT table load (relu/tanh/exp) starts early.
            # All biases are APs (zcol) so the framework emits no const-AP
            # memsets in the preamble (zcol holds garbage this early; the
            # dummy outputs are dead stores into scr).
            scalar.activation(
                out=scr[0:1, 0:1], in_=scr[0:1, 0:1], func=AF.Relu,
                bias=zcol[0:1, 0:1], scale=1.0
            )
            scalar.activation(
                out=scr[0:1, 0:1], in_=scr[0:1, 0:1], func=AF.Tanh,
                bias=zcol[0:1, 0:1], scale=1.0
            )
            scalar.activation(
                out=scr[0:1, 0:1], in_=scr[0:1, 0:1], func=AF.Exp,
                bias=zcol[0:1, 0:1], scale=0.0
            )
            # relu m1 halves (bias via ACT bias operand, fp16 [128,1] AP)
            scalar.wait_ge(s_pe, 2)
            scalar.activation(
                out=h1m1[:], in_=ps1m1[:], func=AF.Relu, bias=b_col(0, 1), scale=1.0
            ).then_inc(s_act)
            scalar.wait_ge(s_pe, 4)
            scalar.activation(
                out=h2m1[:], in_=ps2m1[:], func=AF.Relu, bias=b_col(1, 1), scale=1.0
            ).then_inc(s_act)
            # logvar lives in ps3m0, which closes one matmul earlier than
            # ps3m1 -> the tanh->exp chain starts ~107ns sooner. The mu tanh
            # reads ps3m1 without a wait: k1m1 retires <=110ns after k1m0
            # while tanh_lv+exp occupy scalar >=650ns.
            scalar.wait_ge(s_pe, 5)
            scalar.activation(
                out=lv_ap, in_=ps3m0[:], func=AF.Tanh, bias=zcol, scale=1.0
            ).then_inc(s_act)
            scalar.activation(
                out=iv[:], in_=lv_ap, func=AF.Exp, bias=zcol, scale=-1.0
            ).then_inc(s_act)
            scalar.activation(
                out=mu_ap, in_=ps3m1[:], func=AF.Tanh, bias=zcol, scale=1.0
            ).then_inc(s_act)

        @block.tensor
        def _(tensor):
            # warmup: keep the PE clock ramped while the input DMA flies.
            for _i in range(n_dummies):
                tensor.matmul(psw[:], lhsT=bw[:, 0:128], rhs=bw[:, 0:128],
                              start=True, stop=True)
            for _i in range(n_narrow):
                tensor.matmul(psw[:, 0:32], lhsT=bw[:, 0:128],
                              rhs=bw[:, 0:32], start=True, stop=True)
            # L1: bias comes via the relu, so psum groups start at k0.
            # [bias|x1|W1k0] arrive one HWDGE chunk before W1k1.
            tensor.wait_ge(dw1a, 16)
            tensor.matmul(ps1m0[:], lhsT=w1_k0m0, rhs=x1T[0], start=True, stop=False)
            tensor.matmul(ps1m0[:], lhsT=w1_k1m0, rhs=x1T[1], start=False, stop=True).then_inc(s_pe)
            tensor.matmul(ps1m1[:], lhsT=w1_k0m1, rhs=x1T[0], start=True, stop=False)
            tensor.matmul(ps1m1[:], lhsT=w1_k1m1, rhs=x1T[1], start=False, stop=True).then_inc(s_pe)
            # Prefetch-waits: these DMA semaphores land while PE sits idle
            # waiting for the relus, so waiting here (instead of adjacent to
            # the dependent matmuls) takes the ~110ns/wait sequencer cost
            # off the critical path.
            tensor.wait_ge(dw2k1, 16)
            tensor.wait_ge(dw2, 16)
            # L2: k0 needs h1m0 (DVE), k1 needs h1m1 (ACT)
            tensor.wait_ge(s_dve, 1)
            tensor.matmul(ps2m0[:], lhsT=w_ap(W2_OFF, 0, 0), rhs=h1m0[:], start=True, stop=False)
            tensor.matmul(ps2m1[:], lhsT=w_ap(W2_OFF, 0, 1), rhs=h1m0[:], start=True, stop=False)
            tensor.wait_ge(s_act, 1)
            tensor.matmul(ps2m0[:], lhsT=w2k1[:, 0:128], rhs=h1m1[:], start=False, stop=True).then_inc(s_pe)
            tensor.matmul(ps2m1[:], lhsT=w2k1[:, 128:256], rhs=h1m1[:], start=False, stop=True).then_inc(s_pe)
            # prefetch-waits (all land well before relu2 finishes)
            tensor.wait_ge(dw3, 16)
            tensor.wait_ge(dw3k1, 16)
            tensor.wait_ge(dbrow, 16)
            # The b3 bias matmuls open both L3 psum banks here, in the PE
            # idle window while the relu2 pair runs (the ps2 groups are
            # closed by now, so at most two accumulation groups are open).
            tensor.matmul(ps3m0[:], lhsT=b3m0_row, rhs=ones_row,
                          start=True, stop=False)
            tensor.matmul(ps3m1[:], lhsT=b3m1_row, rhs=ones_row,
                          start=True, stop=False)
            # L3: the four k-matmuls accumulate; m0 (mu) stops first, m1
            # (logvar) right behind.
            tensor.wait_ge(s_dve, 2)
            tensor.matmul(ps3m0[:], lhsT=w_ap(W3_OFF, 0, 0), rhs=h2m0[:], start=False, stop=False)
            tensor.wait_ge(s_act, 2)
            tensor.matmul(ps3m0[:], lhsT=w3k1[:, 0:128], rhs=h2m1[:], start=False, stop=True).then_inc(s_pe)
            tensor.matmul(ps3m1[:], lhsT=w_ap(W3_OFF, 0, 1), rhs=h2m0[:], start=False, stop=False)
            tensor.matmul(ps3m1[:], lhsT=w3k1[:, 128:256], rhs=h2m1[:], start=False, stop=True).then_inc(s_pe)

        @block.vector
        def _(vector):
            # relu m0 halves: (ps + bias) then max 0, one DVE op each
            vector.wait_ge(s_pe, 1)
            vector.tensor_scalar(
                out=h1m0[:], in0=ps1m0[:], scalar1=b_col(0, 0), scalar2=0.0,
                op0=ALU.add, op1=ALU.max,
            ).then_inc(s_dve)
            vector.wait_ge(s_pe, 3)
            vector.tensor_scalar(
                out=h2m0[:], in0=ps2m0[:], scalar1=b_col(1, 0), scalar2=0.0,
                op0=ALU.add, op1=ALU.max,
            ).then_inc(s_dve)
            # prefetch-wait: dab lands long before the tail needs A|B
            vector.wait_ge(dab, 16)
            # tail: c2 = iv*B as soon as exp lands, tmp = mu*A once the mu
            # tanh lands, then c1 = iv*tmp; accumulators give the
            # per-partition sums.
            vector.wait_ge(s_act, 4)
            vector.scalar_tensor_tensor(
                out=scr[:], in0=iv[:], scalar=1.0, in1=B_ap,
                op0=ALU.bypass, op1=ALU.mult, accum_out=out_sb[:, 1:2],
            )
            vector.wait_ge(s_act, 5)
            vector.scalar_tensor_tensor(
                out=tmp[:], in0=mu_ap, scalar=1.0, in1=A_ap,
                op0=ALU.bypass, op1=ALU.mult,
            )
            vector.scalar_tensor_tensor(
                out=scr[:], in0=iv[:], scalar=1.0, in1=tmp[:],
                op0=ALU.bypass, op1=ALU.mult, accum_out=out_sb[:, 0:1],
            ).then_inc(s_dve)

    _split_multi_waits(nc)
    return nc


def _split_multi_waits(nc):
    """This walrus build encodes at most one sync-wait per instruction.
    Hoist extra waits onto same-engine NoOps immediately preceding the
    instruction (engines execute their stream in order, so this is
    semantically identical)."""
    for fn in nc.m.functions:
        for bb in fn.blocks:
            new_insts = []
            for ins in bb.instructions:
                si = ins.sync_info
                if si is not None and len(si.on_wait) > 1:
                    waits = list(si.on_wait)
                    for j, w in enumerate(waits[:-1]):
                        nop = mybir.InstNoOp(
                            name=f"{ins.name}-sw{j}",
                            sync_info=mybir.SyncInfo(on_wait=[w], on_update=[]),
                            bass_nofuse=True,
                            engine=ins.engine,
                        )
                        new_insts.append(nop)
                    si.on_wait = [waits[-1]]
                new_insts.append(ins)
            if len(new_insts) != len(bb.instructions):
                bb.instructions[:] = new_insts


def _pack_inputs(x1, x2, W1, b1, W2, b2, W3, b3):
    f32, f16 = np.float32, np.float16

    def wsec(W):
        W = np.ascontiguousarray(W, f32)
        s = np.empty((P, 512), f16)
        for k in range(2):
            for m in range(2):
                s[:, k * 256 + m * 128 : k * 256 + (m + 1) * 128] = W[
                    k * 128 : (k + 1) * 128, m * 128 : (m + 1) * 128
                ].astype(f16)
        return s

    w1s, w2s, w3s = wsec(W1), wsec(W2), wsec(W3)
    b1 = np.asarray(b1, f32)
    b2 = np.asarray(b2, f32)
    b3 = np.asarray(b3, f32)
    brow = np.zeros((1, 384), f16)
    brow[0, 0:128] = b3[128:256].astype(f16)   # logvar half -> ps3m0
    brow[0, 128:256] = b3[0:128].astype(f16)   # mu half -> ps3m1
    brow[0, 256:384] = 1.0
    x2f = np.asarray(x2, np.float64)
    m1 = x2f.mean(0)
    m2 = (x2f * x2f).mean(0)
    in_maps = []
    for c in range(NCORES):
        bw = np.empty((P, BW_W), f16)
        x1s = np.asarray(x1[c * ROWS : (c + 1) * ROWS], f32)
        x2s = np.asarray(x2[c * ROWS : (c + 1) * ROWS], np.float64)
        bw_u16 = bw.view(np.uint16)
        for li, b in enumerate((b1, b2)):
            for m in range(2):
                c = (2 * li + m) * 2
                bw_u16[:, c : c + 2] = (
                    b[m * 128 : (m + 1) * 128].astype(f32).view(np.uint16).reshape(P, 2)
                )
        bw_u16[:, 8:10] = 0
        bw_u16[:, 10:16] = 0
        bw[:, 16:144] = x1s[:, 0:128].T.astype(f16)
        bw[:, 144:272] = w1s[:, 0:128]      # W1 k0 m0
        bw[:, 272:400] = x1s[:, 128:256].T.astype(f16)
        bw[:, 400:528] = w1s[:, 256:384]    # W1 k1 m0
        bw[:, 528:656] = w1s[:, 128:256]    # W1 k0 m1
        bw[:, 656:784] = w1s[:, 384:512]    # W1 k1 m1
        bw[:, W2_OFF:W3_OFF] = w2s[:, 0:256]
        w3k0_sw = np.concatenate(
            [w3s[:, 128:256], w3s[:, 0:128]], axis=1)      # [lv|mu] k0
        bw[:, W3_OFF:BW_W] = w3k0_sw
        w2k1a = np.ascontiguousarray(w2s[:, 256:512])
        w3k1a = np.concatenate(
            [w3s[:, 384:512], w3s[:, 256:384]], axis=1)    # [lv|mu] k1
        abb = np.empty((P, 256), f32)
        abb[:, 0:ROWS] = (x2s - m1).T.astype(f32)
        abb[:, ROWS : 2 * ROWS] = (0.5 * (x2s * x2s - m2)).T.astype(f32)
        in_maps.append(
            {"blobW": bw, "blobW2K1": w2k1a, "blobW3K1": w3k1a,
             "brow": brow, "blobAB": abb}
        )
    return in_maps


def _run(in_maps, **kwargs):
    global _module_cache
    if _module_cache is None:
        _module_cache = _build_module()
    return run_bass_kernel_spmd(
        _module_cache, in_maps, core_ids=list(range(NCORES)), **kwargs
    )


def _combine(results):
    tot = 0.0
    for r in results:
        o = np.asarray(r["out"], np.float64)
        tot += float(np.sum(o[:, 0] - o[:, 1]))
    return np.float32(tot / N)


def kernel(x1, x2, W1, b1, W2, b2, W3, b3):
    in_maps = _pack_inputs(x1, x2, W1, b1, W2, b2, W3, b3)
    res = _run(in_maps)
    return _combine(res.results)
